# revision 28
# baseline (speedup 1.0000x reference)
"""DigitCapsules dynamic-routing kernel for 8 TRN2 NeuronCores.

Strategy (hardcoded for B=128, R=2048, O=16, D=16, C=16, 3 routing iters):
  - Shard R across the 8 cores (256 routes/core); x replicated.
  - u_hat (= x @ W) generated once on the TensorEngine (K=16 matmuls packed
    4x via row tile_position) and kept SBUF-resident as f16
    [b=128 partitions, (o, c, r)] with r innermost (dense for DVE 2x mode).
  - Iteration 0 uses uniform c_ij, so s0 = x @ (sum_o W)/R comes straight
    from a second tiny matmul against the o-reduced weights (Wbar).
  - Routing contractions over O (weighted by c_ij) and over C (agreement)
    are strided pairwise tree-adds in f16 (DVE 2x mode); 3 of 16 r-chunks
    go to GpSimd to offload the VectorEngine.
  - softmax over global R: b_ij stays in [-0.14, 0.35] so no max pass;
    cross-core denominator = one 8 KB AllReduce per iteration (iters 1,2).
  - Output v is returned per-core as [b, c, r_loc] f32, assembled on host.
"""

import os
import sys

import numpy as np

for _p in ("/opt/trn_rl_repo", "/root/.axon_site/_ro/trn_rl_repo"):
    if os.path.isdir(_p) and _p not in sys.path:
        sys.path.insert(0, _p)

import concourse.bass as bass  # noqa: E402
from concourse import bacc  # noqa: E402
import concourse.tile as tile  # noqa: E402
from concourse import mybir  # noqa: E402
from concourse import bass_utils  # noqa: E402

B, R, O, D, C = 128, 2048, 16, 16, 16
NCORES = 8
RLOC = R // NCORES  # 256
NG = 4  # d-groups at partition offsets 0/32/64/96 (r-interleaved: r % 4 == g)
RG = RLOC // NG  # 64 r's per group
NW = 4  # gen windows; window h covers contiguous global r in [h*64, (h+1)*64)
RW = RG // NW  # 16 r's per (group, window)
RCH = 16  # r chunk size in routing phase
NCH = RLOC // RCH  # 16
GPSIMD_CHUNKS = 0  # chunks routed to GpSimd instead of DVE (measured: net loss)
ROUTING_ITERS = 3
F16 = mybir.dt.float16
F32 = mybir.dt.float32

LAST_EXEC_NS = None
_NC_CACHE = {}


TAGS16 = {"P": "P", 8: "T8", 4: "T4", 2: "T2", "a": "T1"}
TAGS32 = {"P": "Px", 8: "P", 4: "T8", 2: "T4", "a": "T2"}


def _tree_o(nc, eng, scr, lvl, rch, dst_final, tags):
    """Sum over dim1 (size O) of [128, O, C, rch]; final level written to
    dst_final [128, C, rch]."""
    cnt = O
    while cnt > 2:
        half = cnt // 2
        dst = scr.tile([128, half, C, rch], F16, tag=tags[half], name=f"To{half}")
        pv = lvl.rearrange("p (o2 t) c r -> p o2 t c r", t=2)
        eng.tensor_add(dst, pv[:, :, 0], pv[:, :, 1])
        lvl = dst
        cnt = half
    pv = lvl.rearrange("p (o2 t) c r -> p o2 t c r", t=2)
    eng.tensor_add(dst_final, pv[:, 0, 0], pv[:, 0, 1])


def _tree_c(nc, eng, scr, lvl, rch, dst_final, accumulate, tags):
    """Sum over dim2 (size C) of [128, O, C, rch]; final written (or added)
    into dst_final [128, O, rch]."""
    cnt = C
    while cnt > 2:
        half = cnt // 2
        dst = scr.tile([128, O, half, rch], F16, tag=tags[half], name=f"Tc{half}")
        pv = lvl.rearrange("p o (c2 t) r -> p o c2 t r", t=2)
        eng.tensor_add(dst, pv[:, :, :, 0], pv[:, :, :, 1])
        lvl = dst
        cnt = half
    pv = lvl.rearrange("p o (c2 t) r -> p o c2 t r", t=2)
    if accumulate:
        a_ch = scr.tile([128, O, rch], F16, tag=tags["a"], name="a_ch")
        eng.tensor_add(a_ch, pv[:, :, 0, 0], pv[:, :, 0, 1])
        eng.tensor_add(dst_final, dst_final, a_ch)
    else:
        eng.tensor_add(dst_final, pv[:, :, 0, 0], pv[:, :, 0, 1])


def _spass_chunk(nc, eng, scr, u, e_t, s_full, r0, rch, tags=TAGS16):
    rs = slice(r0, r0 + rch)
    P = scr.tile([128, O, C, rch], F16, tag=tags["P"], name="P")
    cb = e_t[:, :, rs].unsqueeze(2).broadcast_to([128, O, C, rch])
    eng.tensor_mul(P, u[:, :, :, rs], cb)
    _tree_o(nc, eng, scr, P, rch, s_full[:, :, rs], tags)


def _squash_range(nc, scr, s_full, ns_t, rt_t, rtf, r0, rlen):
    """In-place squash of s_full[:, :, r0:r0+rlen]: v = s*sqrt(ns)/(1+ns),
    ns = sum_c s^2 (f16 tree, f32 tail)."""
    rs = slice(r0, r0 + rlen)
    s = s_full[:, :, rs]
    sq = scr.tile([128, C, rlen], F16, tag="P", name="sq")
    nc.vector.tensor_mul(sq, s, s)
    lvl = sq
    cnt = C
    while cnt > 2:
        half = cnt // 2
        dst = scr.tile([128, half, rlen], F16, tag=f"T{half}", name=f"q{half}")
        pv = lvl.rearrange("p (c2 t) r -> p c2 t r", t=2)
        nc.vector.tensor_add(dst, pv[:, :, 0], pv[:, :, 1])
        lvl = dst
        cnt = half
    pv = lvl.rearrange("p (c2 t) r -> p c2 t r", t=2)
    ns = ns_t[:, rs]
    rt = rt_t[:, rs]
    rtfs = rtf[:, rs]
    nc.vector.tensor_add(ns, pv[:, 0, 0], pv[:, 0, 1])
    nc.scalar.sqrt(rt, ns)
    nc.vector.tensor_scalar_add(ns, ns, 1.0)
    nc.vector.reciprocal(ns, ns)
    nc.vector.tensor_mul(rt, rt, ns)  # rt = factor (f32)
    nc.scalar.copy(rtfs, rt)
    nc.vector.tensor_mul(s, s, rtfs.unsqueeze(1).broadcast_to([128, C, rlen]))


def _apass_chunk(nc, eng, scr, u, v, b_t, r0, rch, accumulate, tags=TAGS16):
    rs = slice(r0, r0 + rch)
    P2 = scr.tile([128, O, C, rch], F16, tag=tags["P"], name="P2")
    vb = v[:, :, rs].unsqueeze(1).broadcast_to([128, O, C, rch])
    eng.tensor_mul(P2, u[:, :, :, rs], vb)
    _tree_c(nc, eng, scr, P2, rch, b_t[:, :, rs], accumulate, tags)


def _build_nc():
    nc = bacc.Bacc(
        "TRN2",
        target_bir_lowering=False,
        debug=False,
        enable_asserts=False,
        num_devices=NCORES,
    )
    xt_d = nc.dram_tensor("xt", [D, B], F32, kind="ExternalInput")
    w_d = nc.dram_tensor("w", [NG * D, O, C, RG], F32, kind="ExternalInput")
    out_d = nc.dram_tensor("out", [B, C, RLOC], F32, kind="ExternalOutput")

    with tile.TileContext(nc) as tc:
        _body(tc, xt_d.ap(), w_d.ap(), out_d.ap())
    nc.compile()
    return nc


def _body(tc, xt_ap, w_ap, out_ap):
    nc = tc.nc
    with (
        tc.tile_pool(name="const", bufs=1) as constp,
        tc.tile_pool(name="upool", bufs=1) as upool,
        tc.tile_pool(name="state", bufs=1) as st,
        tc.tile_pool(name="scr", bufs=1) as scr,
        tc.tile_pool(name="ccdram", bufs=2, space="DRAM") as dramp,
    ):
        xt16 = constp.tile([128, B], F16)
        u = upool.tile([128, O, C, RLOC], F16)
        s_full = st.tile([128, C, RLOC], F16)  # holds s, then v in place
        b_t = st.tile([128, O, RLOC], F16)
        ns_t = st.tile([128, RLOC], F32)
        rt_t = st.tile([128, RLOC], F32)
        rtf = st.tile([128, RLOC], F16)
        zl = st.tile([128, O], F32)
        zg = st.tile([128, O], F32)
        zgf = st.tile([128, O], F16)
        e_t = st.tile([128, O, RLOC], F16)  # exp(b), then c_ij in place

        # ---- generation (u = x@W, s0 = x@Wbar/R) interleaved with iter 0 ----
        # Group g holds global routes r with r % 4 == g, so each window h
        # completes the contiguous range [h*64, (h+1)*64) and iteration 0's
        # squash + agreement for that range overlaps later windows' matmuls.
        for g in range(NG):
            nc.gpsimd.dma_start(out=xt16[32 * g : 32 * g + D, :], in_=xt_ap)

        # view of u / s_full with the (rq, g) split: r_global = 4*rq + g
        u_il = u.rearrange("p o c (rq g4) -> p o c rq g4", g4=NG)
        s_il = s_full.rearrange("p c (rq g4) -> p c rq g4", g4=NG)

        with (
            tc.tile_pool(name="wpool", bufs=1) as wpool,
            tc.tile_pool(name="psum", bufs=8, space="PSUM") as psump,
        ):
            wbar = wpool.tile([128, C, RG], F16, tag="wbar")
            for h in range(NW):
                wch = wpool.tile([128, O, C, RW], F16, tag="w", name=f"w{h}")
                for g in range(NG):
                    nc.gpsimd.dma_start(
                        out=wch[32 * g : 32 * g + D],
                        in_=w_ap[g * D : (g + 1) * D, :, :, h * RW : (h + 1) * RW],
                    )
                rq = slice(h * RW, (h + 1) * RW)
                # u MMs for this window (2 o's per matmul -> N=512, one bank)
                for g in range(NG):
                    lhsT = xt16[32 * g : 32 * g + D, :]
                    for o2 in range(O // 2):
                        ps = psump.tile(
                            [128, 2, C, RW], F32, tag="ps", name=f"ps{h}_{g}_{o2}"
                        )
                        nc.tensor.matmul(
                            ps,
                            lhsT,
                            wch[32 * g : 32 * g + D, 2 * o2 : 2 * o2 + 2],
                            start=True,
                            stop=True,
                            tile_position=(32 * g, 0),
                        )
                        dst = u_il[:, 2 * o2 : 2 * o2 + 2, :, rq, g]
                        # Split PSUM->SBUF drain between DVE and ScalarE
                        if o2 % 4 == 3:
                            nc.vector.tensor_copy(dst, ps)
                        else:
                            nc.scalar.copy(dst, ps)
                # Wbar = sum_o W for this window (tree over o on DVE).
                # Scratch reuses the routing P/T slots (same sizes).
                lvl = wch
                cnt = O
                while cnt > 2:
                    half = cnt // 2
                    tag = "P" if cnt == O else f"T{2 * half}"
                    dst = scr.tile([128, half, C, RW], F16, tag=tag, name=f"Wb{half}")
                    pv = lvl.rearrange("p (o2 t) c r -> p o2 t c r", t=2)
                    nc.vector.tensor_add(dst, pv[:, :, 0], pv[:, :, 1])
                    lvl = dst
                    cnt = half
                pv = lvl.rearrange("p (o2 t) c r -> p o2 t c r", t=2)
                nc.vector.tensor_add(
                    wbar[:, :, h * RW : (h + 1) * RW], pv[:, 0, 0], pv[:, 0, 1]
                )
                # s0 (unscaled) = x @ wbar for this window
                for g in range(NG):
                    ps0 = psump.tile([128, C, RW], F32, tag="ps", name=f"ps0{h}_{g}")
                    nc.tensor.matmul(
                        ps0,
                        xt16[32 * g : 32 * g + D, :],
                        wbar[32 * g : 32 * g + D, :, h * RW : (h + 1) * RW],
                        start=True,
                        stop=True,
                        tile_position=(32 * g, 0),
                    )
                    nc.scalar.mul(s_il[:, :, rq, g], ps0, 1.0 / R)

                # iteration-0 work for this window's contiguous r range;
                # exp(b) for the next iteration hides under the agreement pass
                w0 = h * (RLOC // NW)
                _squash_range(nc, scr, s_full, ns_t, rt_t, rtf, w0, RLOC // NW)
                for ch in range(w0 // RCH, (w0 + RLOC // NW) // RCH):
                    _apass_chunk(
                        nc, nc.vector, scr, u, s_full, b_t, ch * RCH, RCH, False
                    )
                    rs = slice(ch * RCH, (ch + 1) * RCH)
                    nc.scalar.activation(
                        e_t[:, :, rs], b_t[:, :, rs],
                        mybir.ActivationFunctionType.Exp,
                    )

        # ---------------- routing iterations 1..2 ----------------
        nc.vector.tensor_reduce(
            zl, e_t, axis=mybir.AxisListType.X, op=mybir.AluOpType.add
        )

        for it in range(1, ROUTING_ITERS):
            # c_ij = exp(b) / allreduce(sum_r exp(b)); |b| < 1 so no max.
            # (e_t and zl were computed under the previous agreement pass.)
            cc_in = dramp.tile([128, O], F32, name=f"cc_in{it}")
            cc_out = dramp.tile([128, O], F32, name=f"cc_out{it}")
            nc.gpsimd.dma_start(out=cc_in, in_=zl)
            nc.gpsimd.collective_compute(
                "AllReduce",
                mybir.AluOpType.add,
                replica_groups=[list(range(NCORES))],
                ins=[cc_in.opt()],
                outs=[cc_out.opt()],
            )
            nc.gpsimd.dma_start(out=zg, in_=cc_out)
            nc.vector.reciprocal(zg, zg)
            nc.scalar.copy(zgf, zg)
            nc.vector.tensor_mul(
                e_t, e_t, zgf.unsqueeze(2).broadcast_to([128, O, RLOC])
            )

            if it < ROUTING_ITERS - 1:
                # s-pass, squash, then agreement pass: b += sum_c u*v
                for ch in range(NCH // 2):
                    _spass_chunk(
                        nc, nc.vector, scr, u, e_t, s_full, ch * 32, 32, TAGS32
                    )
                _squash_range(nc, scr, s_full, ns_t, rt_t, rtf, 0, RLOC)
                for ch in range(NCH // 2):
                    _apass_chunk(
                        nc, nc.vector, scr, u, s_full, b_t, ch * 32, 32, True, TAGS32
                    )
                    # hide next iteration's exp(b) under the agreement pass
                    rs = slice(ch * 32, (ch + 1) * 32)
                    nc.scalar.activation(
                        e_t[:, :, rs], b_t[:, :, rs],
                        mybir.ActivationFunctionType.Exp,
                    )
                nc.vector.tensor_reduce(
                    zl, e_t, axis=mybir.AxisListType.X, op=mybir.AluOpType.add
                )
            else:
                # final iteration: squash + output DMA streamed per quarter
                rq4 = RLOC // 4
                for q in range(4):
                    for ch in range(q * 2, (q + 1) * 2):
                        _spass_chunk(
                            nc, nc.vector, scr, u, e_t, s_full, ch * 32, 32, TAGS32
                        )
                    r0 = q * rq4
                    _squash_range(nc, scr, s_full, ns_t, rt_t, rtf, r0, rq4)
                    nc.gpsimd.dma_start(
                        out=out_ap[:, :, r0 : r0 + rq4],
                        in_=s_full[:, :, r0 : r0 + rq4],
                    )


def _prep_inputs(x, route_weights):
    xt = np.ascontiguousarray(x.reshape(B, D).T.astype(np.float32))  # [D, B]
    w0 = np.asarray(route_weights).reshape(R, O, D, C)
    in_maps = []
    for i in range(NCORES):
        ws = w0[i * RLOC : (i + 1) * RLOC]  # (RLOC, O, D, C); r = 4*rq + g
        ws = ws.reshape(RG, NG, O, D, C).transpose(1, 3, 2, 4, 0)  # (g, d, o, c, rq)
        wprep = np.ascontiguousarray(ws.reshape(NG * D, O, C, RG).astype(np.float32))
        in_maps.append({"xt": xt, "w": wprep})
    return in_maps


def kernel(x, route_weights, trace=False):
    global LAST_EXEC_NS
    x = np.asarray(x, dtype=np.float32)
    route_weights = np.asarray(route_weights, dtype=np.float32)

    if "nc" not in _NC_CACHE:
        _NC_CACHE["nc"] = _build_nc()
    nc = _NC_CACHE["nc"]

    in_maps = _prep_inputs(x, route_weights)
    res = bass_utils.run_bass_kernel_spmd(
        nc, in_maps, core_ids=list(range(NCORES)), trace=trace
    )
    LAST_EXEC_NS = res.exec_time_ns

    shards = []
    for i in range(NCORES):
        o = res.results[i]["out"]  # [B, C, RLOC]
        shards.append(np.transpose(o, (0, 2, 1)))  # [B, RLOC, C]
    return np.concatenate(shards, axis=1).astype(np.float32)  # (B, R, C)


# revision 31
# speedup vs baseline: 1.0070x; 1.0070x over previous
"""DigitCapsules dynamic-routing kernel for 8 TRN2 NeuronCores.

Strategy (hardcoded for B=128, R=2048, O=16, D=16, C=16, 3 routing iters):
  - Shard R across the 8 cores (256 routes/core); x replicated.
  - u_hat (= x @ W) generated once on the TensorEngine (K=16 matmuls packed
    4x via row tile_position) and kept SBUF-resident as f16
    [b=128 partitions, (o, c, r)] with r innermost (dense for DVE 2x mode).
  - Iteration 0 uses uniform c_ij, so s0 = x @ (sum_o W)/R comes straight
    from a second tiny matmul against the o-reduced weights (Wbar).
  - Routing contractions over O (weighted by c_ij) and over C (agreement)
    are strided pairwise tree-adds in f16 (DVE 2x mode); 3 of 16 r-chunks
    go to GpSimd to offload the VectorEngine.
  - softmax over global R: b_ij stays in [-0.14, 0.35] so no max pass;
    cross-core denominator = one 8 KB AllReduce per iteration (iters 1,2).
  - Output v is returned per-core as [b, c, r_loc] f32, assembled on host.
"""

import os
import sys

import numpy as np

for _p in ("/opt/trn_rl_repo", "/root/.axon_site/_ro/trn_rl_repo"):
    if os.path.isdir(_p) and _p not in sys.path:
        sys.path.insert(0, _p)

import concourse.bass as bass  # noqa: E402
from concourse import bacc  # noqa: E402
import concourse.tile as tile  # noqa: E402
from concourse import mybir  # noqa: E402
from concourse import bass_utils  # noqa: E402

B, R, O, D, C = 128, 2048, 16, 16, 16
NCORES = 8
RLOC = R // NCORES  # 256
NG = 4  # d-groups at partition offsets 0/32/64/96 (r-interleaved: r % 4 == g)
RG = RLOC // NG  # 64 r's per group
NW = 4  # gen windows; window h covers contiguous global r in [h*64, (h+1)*64)
RW = RG // NW  # 16 r's per (group, window)
RCH = 16  # r chunk size in routing phase
NCH = RLOC // RCH  # 16
GPSIMD_CHUNKS = 0  # chunks routed to GpSimd instead of DVE (measured: net loss)
ROUTING_ITERS = 3
F16 = mybir.dt.float16
F32 = mybir.dt.float32

LAST_EXEC_NS = None
_NC_CACHE = {}


def _tree_o(nc, eng, scr, lvl, rch, dst_final, tagsfx=""):
    """Sum over dim1 (size O) of [128, O, C, rch]; final level written to
    dst_final [128, C, rch]."""
    cnt = O
    while cnt > 2:
        half = cnt // 2
        dst = scr.tile(
            [128, half, C, rch], F16, tag=f"T{half}{tagsfx}", name=f"To{half}"
        )
        pv = lvl.rearrange("p (o2 t) c r -> p o2 t c r", t=2)
        eng.tensor_add(dst, pv[:, :, 0], pv[:, :, 1])
        lvl = dst
        cnt = half
    pv = lvl.rearrange("p (o2 t) c r -> p o2 t c r", t=2)
    eng.tensor_add(dst_final, pv[:, 0, 0], pv[:, 0, 1])


def _tree_c(nc, eng, scr, lvl, rch, dst_final, accumulate, tagsfx=""):
    """Sum over dim2 (size C) of [128, O, C, rch]; final written (or added)
    into dst_final [128, O, rch]."""
    cnt = C
    while cnt > 2:
        half = cnt // 2
        dst = scr.tile(
            [128, O, half, rch], F16, tag=f"T{half}{tagsfx}", name=f"Tc{half}"
        )
        pv = lvl.rearrange("p o (c2 t) r -> p o c2 t r", t=2)
        eng.tensor_add(dst, pv[:, :, :, 0], pv[:, :, :, 1])
        lvl = dst
        cnt = half
    pv = lvl.rearrange("p o (c2 t) r -> p o c2 t r", t=2)
    if accumulate:
        a_ch = scr.tile([128, O, rch], F16, tag=f"T1{tagsfx}", name="a_ch")
        eng.tensor_add(a_ch, pv[:, :, 0, 0], pv[:, :, 0, 1])
        eng.tensor_add(dst_final, dst_final, a_ch)
    else:
        eng.tensor_add(dst_final, pv[:, :, 0, 0], pv[:, :, 0, 1])


def _spass_chunk(nc, eng, scr, u, e_t, s_full, r0, rch, tagsfx=""):
    rs = slice(r0, r0 + rch)
    P = scr.tile([128, O, C, rch], F16, tag=f"P{tagsfx}", name="P")
    cb = e_t[:, :, rs].unsqueeze(2).broadcast_to([128, O, C, rch])
    eng.tensor_mul(P, u[:, :, :, rs], cb)
    _tree_o(nc, eng, scr, P, rch, s_full[:, :, rs], tagsfx)


def _squash_range(nc, scr, s_full, ns_t, rt_t, rtf, r0, rlen):
    """In-place squash of s_full[:, :, r0:r0+rlen]: v = s*sqrt(ns)/(1+ns),
    ns = sum_c s^2 (f16 tree, f32 tail)."""
    rs = slice(r0, r0 + rlen)
    s = s_full[:, :, rs]
    sq = scr.tile([128, C, rlen], F16, tag="P", name="sq")
    nc.vector.tensor_mul(sq, s, s)
    lvl = sq
    cnt = C
    while cnt > 2:
        half = cnt // 2
        dst = scr.tile([128, half, rlen], F16, tag=f"T{half}", name=f"q{half}")
        pv = lvl.rearrange("p (c2 t) r -> p c2 t r", t=2)
        nc.vector.tensor_add(dst, pv[:, :, 0], pv[:, :, 1])
        lvl = dst
        cnt = half
    pv = lvl.rearrange("p (c2 t) r -> p c2 t r", t=2)
    ns = ns_t[:, rs]
    rt = rt_t[:, rs]
    rtfs = rtf[:, rs]
    nc.vector.tensor_add(ns, pv[:, 0, 0], pv[:, 0, 1])
    nc.scalar.sqrt(rt, ns)
    nc.vector.tensor_scalar_add(ns, ns, 1.0)
    nc.vector.reciprocal(ns, ns)
    nc.vector.tensor_mul(rtfs, rt, ns)  # factor, cast to f16 on write
    nc.vector.tensor_mul(s, s, rtfs.unsqueeze(1).broadcast_to([128, C, rlen]))


def _apass_chunk(nc, eng, scr, u, v, b_t, r0, rch, accumulate, tagsfx=""):
    rs = slice(r0, r0 + rch)
    P2 = scr.tile([128, O, C, rch], F16, tag=f"P{tagsfx}", name="P2")
    vb = v[:, :, rs].unsqueeze(1).broadcast_to([128, O, C, rch])
    eng.tensor_mul(P2, u[:, :, :, rs], vb)
    _tree_c(nc, eng, scr, P2, rch, b_t[:, :, rs], accumulate, tagsfx)


def _build_nc():
    nc = bacc.Bacc(
        "TRN2",
        target_bir_lowering=False,
        debug=False,
        enable_asserts=False,
        num_devices=NCORES,
    )
    xt_d = nc.dram_tensor("xt", [D, B], F32, kind="ExternalInput")
    w_d = nc.dram_tensor("w", [NG * D, O, C, RG], F32, kind="ExternalInput")
    out_d = nc.dram_tensor("out", [B, C, RLOC], F32, kind="ExternalOutput")

    with (
        tile.TileContext(nc) as tc,
        nc.allow_low_precision(reason="f16 routing intermediates are intentional"),
    ):
        _body(tc, xt_d.ap(), w_d.ap(), out_d.ap())
    nc.compile()
    return nc


def _body(tc, xt_ap, w_ap, out_ap):
    nc = tc.nc
    with (
        tc.tile_pool(name="const", bufs=1) as constp,
        tc.tile_pool(name="upool", bufs=1) as upool,
        tc.tile_pool(name="state", bufs=1) as st,
        tc.tile_pool(name="scr", bufs=1) as scr,
        tc.tile_pool(name="ccdram", bufs=2, space="DRAM") as dramp,
    ):
        xt16 = constp.tile([128, B], F16)
        u = upool.tile([128, O, C, RLOC], F16)
        s_full = st.tile([128, C, RLOC], F16)  # holds s, then v in place
        b_t = st.tile([128, O, RLOC], F16)
        ns_t = st.tile([128, RLOC], F32)
        rt_t = st.tile([128, RLOC], F32)
        rtf = st.tile([128, RLOC], F16)
        zl = st.tile([128, O], F32)
        zg = st.tile([128, O], F32)
        zgf = st.tile([128, O], F16)
        zlp = st.tile([128, O], F32)
        e_t = st.tile([128, O, RLOC], F16)  # exp(b), then c_ij in place

        # ---- generation (u = x@W, s0 = x@Wbar/R) interleaved with iter 0 ----
        # Group g holds global routes r with r % 4 == g, so each window h
        # completes the contiguous range [h*64, (h+1)*64) and iteration 0's
        # squash + agreement for that range overlaps later windows' matmuls.
        for g in range(NG):
            nc.gpsimd.dma_start(out=xt16[32 * g : 32 * g + D, :], in_=xt_ap)

        # view of u / s_full with the (rq, g) split: r_global = 4*rq + g
        u_il = u.rearrange("p o c (rq g4) -> p o c rq g4", g4=NG)
        s_il = s_full.rearrange("p c (rq g4) -> p c rq g4", g4=NG)

        with (
            tc.tile_pool(name="wpool", bufs=1) as wpool,
            tc.tile_pool(name="psum", bufs=8, space="PSUM") as psump,
        ):
            wbar = wpool.tile([128, C, RG], F16, tag="wbar")
            for h in range(NW):
                wch = wpool.tile([128, O, C, RW], F16, tag="w", name=f"w{h}")
                for g in range(NG):
                    nc.gpsimd.dma_start(
                        out=wch[32 * g : 32 * g + D],
                        in_=w_ap[g * D : (g + 1) * D, :, :, h * RW : (h + 1) * RW],
                    )
                rq = slice(h * RW, (h + 1) * RW)
                # u MMs for this window (2 o's per matmul -> N=512, one bank)
                for g in range(NG):
                    lhsT = xt16[32 * g : 32 * g + D, :]
                    for o2 in range(O // 2):
                        ps = psump.tile(
                            [128, 2, C, RW], F32, tag="ps", name=f"ps{h}_{g}_{o2}"
                        )
                        nc.tensor.matmul(
                            ps,
                            lhsT,
                            wch[32 * g : 32 * g + D, 2 * o2 : 2 * o2 + 2],
                            start=True,
                            stop=True,
                            tile_position=(32 * g, 0),
                        )
                        dst = u_il[:, 2 * o2 : 2 * o2 + 2, :, rq, g]
                        # Split PSUM->SBUF drain between DVE and ScalarE
                        if o2 % 4 == 3:
                            nc.vector.tensor_copy(dst, ps)
                        else:
                            nc.scalar.copy(dst, ps)
                # Wbar = sum_o W for this window (tree over o on DVE).
                # Scratch reuses the routing P/T slots (same sizes).
                lvl = wch
                cnt = O
                while cnt > 2:
                    half = cnt // 2
                    tag = "P" if cnt == O else f"T{2 * half}"
                    dst = scr.tile([128, half, C, RW], F16, tag=tag, name=f"Wb{half}")
                    pv = lvl.rearrange("p (o2 t) c r -> p o2 t c r", t=2)
                    nc.vector.tensor_add(dst, pv[:, :, 0], pv[:, :, 1])
                    lvl = dst
                    cnt = half
                pv = lvl.rearrange("p (o2 t) c r -> p o2 t c r", t=2)
                nc.vector.tensor_add(
                    wbar[:, :, h * RW : (h + 1) * RW], pv[:, 0, 0], pv[:, 0, 1]
                )
                # s0 (unscaled) = x @ wbar for this window
                for g in range(NG):
                    ps0 = psump.tile([128, C, RW], F32, tag="ps", name=f"ps0{h}_{g}")
                    nc.tensor.matmul(
                        ps0,
                        xt16[32 * g : 32 * g + D, :],
                        wbar[32 * g : 32 * g + D, :, h * RW : (h + 1) * RW],
                        start=True,
                        stop=True,
                        tile_position=(32 * g, 0),
                    )
                    nc.scalar.mul(s_il[:, :, rq, g], ps0, 1.0 / R)

                # iteration-0 work for this window's contiguous r range;
                # exp(b) for the next iteration hides under the agreement pass
                w0 = h * (RLOC // NW)
                _squash_range(nc, scr, s_full, ns_t, rt_t, rtf, w0, RLOC // NW)
                for ch in range(w0 // RCH, (w0 + RLOC // NW) // RCH):
                    _apass_chunk(
                        nc, nc.vector, scr, u, s_full, b_t, ch * RCH, RCH, False
                    )
                    rs = slice(ch * RCH, (ch + 1) * RCH)
                    nc.scalar.activation(
                        e_t[:, :, rs], b_t[:, :, rs],
                        mybir.ActivationFunctionType.Exp,
                    )

        # ---------------- routing iterations 1..2 ----------------
        nc.vector.tensor_reduce(
            zl, e_t, axis=mybir.AxisListType.X, op=mybir.AluOpType.add
        )

        for it in range(1, ROUTING_ITERS):
            # c_ij = exp(b) / allreduce(sum_r exp(b)); |b| < 1 so no max.
            # (e_t and zl were computed under the previous agreement pass.)
            cc_in = dramp.tile([128, O], F32, name=f"cc_in{it}")
            cc_out = dramp.tile([128, O], F32, name=f"cc_out{it}")
            nc.gpsimd.dma_start(out=cc_in, in_=zl)
            nc.gpsimd.collective_compute(
                "AllReduce",
                mybir.AluOpType.add,
                replica_groups=[list(range(NCORES))],
                ins=[cc_in.opt()],
                outs=[cc_out.opt()],
            )
            nc.gpsimd.dma_start(out=zg, in_=cc_out)
            nc.vector.reciprocal(zgf, zg)
            nc.vector.tensor_mul(
                e_t, e_t, zgf.unsqueeze(2).broadcast_to([128, O, RLOC])
            )

            if it < ROUTING_ITERS - 1:
                # s-pass, squash, then agreement pass: b += sum_c u*v
                for ch in range(NCH):
                    _spass_chunk(nc, nc.vector, scr, u, e_t, s_full, ch * RCH, RCH)
                _squash_range(nc, scr, s_full, ns_t, rt_t, rtf, 0, RLOC)
                for ch in range(NCH):
                    _apass_chunk(
                        nc, nc.vector, scr, u, s_full, b_t, ch * RCH, RCH, True
                    )
                    # hide next iteration's exp(b) under the agreement pass
                    rs = slice(ch * RCH, (ch + 1) * RCH)
                    nc.scalar.activation(
                        e_t[:, :, rs], b_t[:, :, rs],
                        mybir.ActivationFunctionType.Exp,
                    )
                    if ch == NCH - 3:
                        nc.vector.tensor_reduce(
                            zlp,
                            e_t[:, :, : (NCH - 2) * RCH],
                            axis=mybir.AxisListType.X,
                            op=mybir.AluOpType.add,
                        )
                nc.vector.tensor_reduce(
                    zl,
                    e_t[:, :, (NCH - 2) * RCH :],
                    axis=mybir.AxisListType.X,
                    op=mybir.AluOpType.add,
                )
                nc.vector.tensor_add(zl, zl, zlp)
            else:
                # final iteration: squash + output DMA streamed per quarter
                rq4 = RLOC // 4
                for q in range(4):
                    for ch in range(q * 4, (q + 1) * 4):
                        _spass_chunk(
                            nc, nc.vector, scr, u, e_t, s_full, ch * RCH, RCH
                        )
                    r0 = q * rq4
                    _squash_range(nc, scr, s_full, ns_t, rt_t, rtf, r0, rq4)
                    nc.gpsimd.dma_start(
                        out=out_ap[:, :, r0 : r0 + rq4],
                        in_=s_full[:, :, r0 : r0 + rq4],
                    )


def _prep_inputs(x, route_weights):
    xt = np.ascontiguousarray(x.reshape(B, D).T.astype(np.float32))  # [D, B]
    w0 = np.asarray(route_weights).reshape(R, O, D, C)
    in_maps = []
    for i in range(NCORES):
        ws = w0[i * RLOC : (i + 1) * RLOC]  # (RLOC, O, D, C); r = 4*rq + g
        ws = ws.reshape(RG, NG, O, D, C).transpose(1, 3, 2, 4, 0)  # (g, d, o, c, rq)
        wprep = np.ascontiguousarray(ws.reshape(NG * D, O, C, RG).astype(np.float32))
        in_maps.append({"xt": xt, "w": wprep})
    return in_maps


def kernel(x, route_weights, trace=False):
    global LAST_EXEC_NS
    x = np.asarray(x, dtype=np.float32)
    route_weights = np.asarray(route_weights, dtype=np.float32)

    if "nc" not in _NC_CACHE:
        _NC_CACHE["nc"] = _build_nc()
    nc = _NC_CACHE["nc"]

    in_maps = _prep_inputs(x, route_weights)
    res = bass_utils.run_bass_kernel_spmd(
        nc, in_maps, core_ids=list(range(NCORES)), trace=trace
    )
    LAST_EXEC_NS = res.exec_time_ns

    shards = []
    for i in range(NCORES):
        o = res.results[i]["out"]  # [B, C, RLOC]
        shards.append(np.transpose(o, (0, 2, 1)))  # [B, RLOC, C]
    return np.concatenate(shards, axis=1).astype(np.float32)  # (B, R, C)


# revision 33
# speedup vs baseline: 1.0073x; 1.0002x over previous
"""DigitCapsules dynamic-routing kernel for 8 TRN2 NeuronCores.

Strategy (hardcoded for B=128, R=2048, O=16, D=16, C=16, 3 routing iters):
  - Shard R across the 8 cores (256 routes/core); x replicated.
  - u_hat (= x @ W) generated once on the TensorEngine (K=16 matmuls packed
    4x via row tile_position) and kept SBUF-resident as f16
    [b=128 partitions, (o, c, r)] with r innermost (dense for DVE 2x mode).
  - Iteration 0 uses uniform c_ij, so s0 = x @ (sum_o W)/R comes straight
    from a second tiny matmul against the o-reduced weights (Wbar).
  - Routing contractions over O (weighted by c_ij) and over C (agreement)
    are strided pairwise tree-adds in f16 (DVE 2x mode); 3 of 16 r-chunks
    go to GpSimd to offload the VectorEngine.
  - softmax over global R: b_ij stays in [-0.14, 0.35] so no max pass;
    cross-core denominator = one 8 KB AllReduce per iteration (iters 1,2).
  - Output v is returned per-core as [b, c, r_loc] f32, assembled on host.
"""

import os
import sys

import numpy as np

for _p in ("/opt/trn_rl_repo", "/root/.axon_site/_ro/trn_rl_repo"):
    if os.path.isdir(_p) and _p not in sys.path:
        sys.path.insert(0, _p)

import concourse.bass as bass  # noqa: E402
from concourse import bacc  # noqa: E402
import concourse.tile as tile  # noqa: E402
from concourse import mybir  # noqa: E402
from concourse import bass_utils  # noqa: E402

B, R, O, D, C = 128, 2048, 16, 16, 16
NCORES = 8
RLOC = R // NCORES  # 256
NG = 4  # d-groups at partition offsets 0/32/64/96 (r-interleaved: r % 4 == g)
RG = RLOC // NG  # 64 r's per group
NW = 4  # gen windows; window h covers contiguous global r in [h*64, (h+1)*64)
RW = RG // NW  # 16 r's per (group, window)
RCH = 16  # r chunk size in routing phase
NCH = RLOC // RCH  # 16
GPSIMD_CHUNKS = 0  # chunks routed to GpSimd instead of DVE (measured: net loss)
ROUTING_ITERS = 3
F16 = mybir.dt.float16
F32 = mybir.dt.float32

LAST_EXEC_NS = None
_NC_CACHE = {}


def _tree_o(nc, eng, scr, lvl, rch, dst_final, tagsfx=""):
    """Sum over dim1 (size O) of [128, O, C, rch]; final level written to
    dst_final [128, C, rch]."""
    cnt = O
    while cnt > 2:
        half = cnt // 2
        dst = scr.tile(
            [128, half, C, rch], F16, tag=f"T{half}{tagsfx}", name=f"To{half}"
        )
        pv = lvl.rearrange("p (o2 t) c r -> p o2 t c r", t=2)
        eng.tensor_add(dst, pv[:, :, 0], pv[:, :, 1])
        lvl = dst
        cnt = half
    pv = lvl.rearrange("p (o2 t) c r -> p o2 t c r", t=2)
    eng.tensor_add(dst_final, pv[:, 0, 0], pv[:, 0, 1])


def _tree_c(nc, eng, scr, lvl, rch, dst_final, accumulate, tagsfx=""):
    """Sum over dim2 (size C) of [128, O, C, rch]; final written (or added)
    into dst_final [128, O, rch]."""
    cnt = C
    while cnt > 2:
        half = cnt // 2
        dst = scr.tile(
            [128, O, half, rch], F16, tag=f"T{half}{tagsfx}", name=f"Tc{half}"
        )
        pv = lvl.rearrange("p o (c2 t) r -> p o c2 t r", t=2)
        eng.tensor_add(dst, pv[:, :, :, 0], pv[:, :, :, 1])
        lvl = dst
        cnt = half
    pv = lvl.rearrange("p o (c2 t) r -> p o c2 t r", t=2)
    if accumulate:
        a_ch = scr.tile([128, O, rch], F16, tag=f"T1{tagsfx}", name="a_ch")
        eng.tensor_add(a_ch, pv[:, :, 0, 0], pv[:, :, 0, 1])
        eng.tensor_add(dst_final, dst_final, a_ch)
    else:
        eng.tensor_add(dst_final, pv[:, :, 0, 0], pv[:, :, 0, 1])


def _spass_chunk(nc, eng, scr, u, e_t, s_full, r0, rch, tagsfx=""):
    rs = slice(r0, r0 + rch)
    P = scr.tile([128, O, C, rch], F16, tag=f"P{tagsfx}", name="P")
    cb = e_t[:, :, rs].unsqueeze(2).broadcast_to([128, O, C, rch])
    eng.tensor_mul(P, u[:, :, :, rs], cb)
    _tree_o(nc, eng, scr, P, rch, s_full[:, :, rs], tagsfx)


def _squash_range(nc, scr, s_full, ns_t, rt_t, rtf, r0, rlen):
    """In-place squash of s_full[:, :, r0:r0+rlen]: v = s*sqrt(ns)/(1+ns),
    ns = sum_c s^2 (f16 tree, f32 tail)."""
    rs = slice(r0, r0 + rlen)
    s = s_full[:, :, rs]
    sq = scr.tile([128, C, rlen], F16, tag="P", name="sq")
    nc.vector.tensor_mul(sq, s, s)
    lvl = sq
    cnt = C
    while cnt > 2:
        half = cnt // 2
        dst = scr.tile([128, half, rlen], F16, tag=f"T{half}", name=f"q{half}")
        pv = lvl.rearrange("p (c2 t) r -> p c2 t r", t=2)
        nc.vector.tensor_add(dst, pv[:, :, 0], pv[:, :, 1])
        lvl = dst
        cnt = half
    pv = lvl.rearrange("p (c2 t) r -> p c2 t r", t=2)
    ns = ns_t[:, rs]
    rt = rt_t[:, rs]
    rtfs = rtf[:, rs]
    nc.vector.tensor_add(ns, pv[:, 0, 0], pv[:, 0, 1])
    nc.scalar.sqrt(rt, ns)
    nc.vector.tensor_scalar_add(ns, ns, 1.0)
    nc.vector.reciprocal(ns, ns)
    nc.vector.tensor_mul(rt, rt, ns)  # rt = factor (f32)
    nc.scalar.copy(rtfs, rt)
    nc.vector.tensor_mul(s, s, rtfs.unsqueeze(1).broadcast_to([128, C, rlen]))


def _apass_chunk(nc, eng, scr, u, v, b_t, r0, rch, accumulate, tagsfx=""):
    rs = slice(r0, r0 + rch)
    P2 = scr.tile([128, O, C, rch], F16, tag=f"P{tagsfx}", name="P2")
    vb = v[:, :, rs].unsqueeze(1).broadcast_to([128, O, C, rch])
    eng.tensor_mul(P2, u[:, :, :, rs], vb)
    _tree_c(nc, eng, scr, P2, rch, b_t[:, :, rs], accumulate, tagsfx)


def _build_nc():
    nc = bacc.Bacc(
        "TRN2",
        target_bir_lowering=False,
        debug=False,
        enable_asserts=False,
        num_devices=NCORES,
    )
    xt_d = nc.dram_tensor("xt", [D, B], F32, kind="ExternalInput")
    w_d = nc.dram_tensor("w", [NG * D, O, C, RG], F32, kind="ExternalInput")
    out_d = nc.dram_tensor("out", [B, C, RLOC], F32, kind="ExternalOutput")

    with tile.TileContext(nc) as tc:
        _body(tc, xt_d.ap(), w_d.ap(), out_d.ap())
    nc.compile()
    return nc


def _body(tc, xt_ap, w_ap, out_ap):
    nc = tc.nc
    with (
        tc.tile_pool(name="const", bufs=1) as constp,
        tc.tile_pool(name="upool", bufs=1) as upool,
        tc.tile_pool(name="state", bufs=1) as st,
        tc.tile_pool(name="scr", bufs=1) as scr,
        tc.tile_pool(name="ccdram", bufs=2, space="DRAM") as dramp,
    ):
        xt16 = constp.tile([128, B], F16)
        u = upool.tile([128, O, C, RLOC], F16)
        s_full = st.tile([128, C, RLOC], F16)  # holds s, then v in place
        b_t = st.tile([128, O, RLOC], F16)
        ns_t = st.tile([128, RLOC], F32)
        rt_t = st.tile([128, RLOC], F32)
        rtf = st.tile([128, RLOC], F16)
        zl = st.tile([128, O], F32)
        zg = st.tile([128, O], F32)
        zgf = st.tile([128, O], F16)
        e_t = st.tile([128, O, RLOC], F16)  # exp(b), then c_ij in place

        # ---- generation (u = x@W, s0 = x@Wbar/R) interleaved with iter 0 ----
        # Group g holds global routes r with r % 4 == g, so each window h
        # completes the contiguous range [h*64, (h+1)*64) and iteration 0's
        # squash + agreement for that range overlaps later windows' matmuls.
        for g in range(NG):
            nc.gpsimd.dma_start(out=xt16[32 * g : 32 * g + D, :], in_=xt_ap)

        # view of u / s_full with the (rq, g) split: r_global = 4*rq + g
        u_il = u.rearrange("p o c (rq g4) -> p o c rq g4", g4=NG)
        s_il = s_full.rearrange("p c (rq g4) -> p c rq g4", g4=NG)

        with (
            tc.tile_pool(name="wpool", bufs=1) as wpool,
            tc.tile_pool(name="psum", bufs=8, space="PSUM") as psump,
        ):
            wbar = wpool.tile([128, C, RG], F16, tag="wbar")
            for h in range(NW):
                wch = wpool.tile([128, O, C, RW], F16, tag="w", name=f"w{h}", bufs=2)
                for g in range(NG):
                    nc.gpsimd.dma_start(
                        out=wch[32 * g : 32 * g + D],
                        in_=w_ap[g * D : (g + 1) * D, :, :, h * RW : (h + 1) * RW],
                    )
                rq = slice(h * RW, (h + 1) * RW)
                # u MMs for this window (2 o's per matmul -> N=512, one bank)
                for g in range(NG):
                    lhsT = xt16[32 * g : 32 * g + D, :]
                    for o2 in range(O // 2):
                        ps = psump.tile(
                            [128, 2, C, RW], F32, tag="ps", name=f"ps{h}_{g}_{o2}"
                        )
                        nc.tensor.matmul(
                            ps,
                            lhsT,
                            wch[32 * g : 32 * g + D, 2 * o2 : 2 * o2 + 2],
                            start=True,
                            stop=True,
                            tile_position=(32 * g, 0),
                        )
                        dst = u_il[:, 2 * o2 : 2 * o2 + 2, :, rq, g]
                        # Split PSUM->SBUF drain between DVE and ScalarE
                        if o2 % 4 == 3:
                            nc.vector.tensor_copy(dst, ps)
                        else:
                            nc.scalar.copy(dst, ps)
                # Wbar = sum_o W for this window (tree over o on DVE).
                # Scratch reuses the routing P/T slots (same sizes).
                lvl = wch
                cnt = O
                while cnt > 2:
                    half = cnt // 2
                    tag = "P" if cnt == O else f"T{2 * half}"
                    dst = scr.tile([128, half, C, RW], F16, tag=tag, name=f"Wb{half}")
                    pv = lvl.rearrange("p (o2 t) c r -> p o2 t c r", t=2)
                    nc.vector.tensor_add(dst, pv[:, :, 0], pv[:, :, 1])
                    lvl = dst
                    cnt = half
                pv = lvl.rearrange("p (o2 t) c r -> p o2 t c r", t=2)
                nc.vector.tensor_add(
                    wbar[:, :, h * RW : (h + 1) * RW], pv[:, 0, 0], pv[:, 0, 1]
                )
                # s0 (unscaled) = x @ wbar for this window
                for g in range(NG):
                    ps0 = psump.tile([128, C, RW], F32, tag="ps", name=f"ps0{h}_{g}")
                    nc.tensor.matmul(
                        ps0,
                        xt16[32 * g : 32 * g + D, :],
                        wbar[32 * g : 32 * g + D, :, h * RW : (h + 1) * RW],
                        start=True,
                        stop=True,
                        tile_position=(32 * g, 0),
                    )
                    nc.scalar.mul(s_il[:, :, rq, g], ps0, 1.0 / R)

                # iteration-0 work for this window's contiguous r range;
                # exp(b) for the next iteration hides under the agreement pass
                w0 = h * (RLOC // NW)
                _squash_range(nc, scr, s_full, ns_t, rt_t, rtf, w0, RLOC // NW)
                for ch in range(w0 // RCH, (w0 + RLOC // NW) // RCH):
                    _apass_chunk(
                        nc, nc.vector, scr, u, s_full, b_t, ch * RCH, RCH, False
                    )
                    rs = slice(ch * RCH, (ch + 1) * RCH)
                    nc.scalar.activation(
                        e_t[:, :, rs], b_t[:, :, rs],
                        mybir.ActivationFunctionType.Exp,
                    )

        # ---------------- routing iterations 1..2 ----------------
        nc.vector.tensor_reduce(
            zl, e_t, axis=mybir.AxisListType.X, op=mybir.AluOpType.add
        )

        for it in range(1, ROUTING_ITERS):
            # c_ij = exp(b) / allreduce(sum_r exp(b)); |b| < 1 so no max.
            # (e_t and zl were computed under the previous agreement pass.)
            cc_in = dramp.tile([128, O], F32, name=f"cc_in{it}")
            cc_out = dramp.tile([128, O], F32, name=f"cc_out{it}")
            nc.sync.dma_start(out=cc_in, in_=zl)
            nc.gpsimd.collective_compute(
                "AllReduce",
                mybir.AluOpType.add,
                replica_groups=[list(range(NCORES))],
                ins=[cc_in.opt()],
                outs=[cc_out.opt()],
            )
            nc.sync.dma_start(out=zg, in_=cc_out)
            nc.vector.reciprocal(zg, zg)
            nc.scalar.copy(zgf, zg)
            nc.vector.tensor_mul(
                e_t, e_t, zgf.unsqueeze(2).broadcast_to([128, O, RLOC])
            )

            if it < ROUTING_ITERS - 1:
                # s-pass, squash, then agreement pass: b += sum_c u*v
                for ch in range(NCH):
                    _spass_chunk(nc, nc.vector, scr, u, e_t, s_full, ch * RCH, RCH)
                _squash_range(nc, scr, s_full, ns_t, rt_t, rtf, 0, RLOC)
                for ch in range(NCH):
                    _apass_chunk(
                        nc, nc.vector, scr, u, s_full, b_t, ch * RCH, RCH, True
                    )
                    # hide next iteration's exp(b) under the agreement pass
                    rs = slice(ch * RCH, (ch + 1) * RCH)
                    nc.scalar.activation(
                        e_t[:, :, rs], b_t[:, :, rs],
                        mybir.ActivationFunctionType.Exp,
                    )
                nc.vector.tensor_reduce(
                    zl, e_t, axis=mybir.AxisListType.X, op=mybir.AluOpType.add
                )
            else:
                # final iteration: squash + output DMA streamed per quarter
                rq4 = RLOC // 4
                for q in range(4):
                    for ch in range(q * 4, (q + 1) * 4):
                        _spass_chunk(
                            nc, nc.vector, scr, u, e_t, s_full, ch * RCH, RCH
                        )
                    r0 = q * rq4
                    _squash_range(nc, scr, s_full, ns_t, rt_t, rtf, r0, rq4)
                    nc.gpsimd.dma_start(
                        out=out_ap[:, :, r0 : r0 + rq4],
                        in_=s_full[:, :, r0 : r0 + rq4],
                    )


def _prep_inputs(x, route_weights):
    xt = np.ascontiguousarray(x.reshape(B, D).T.astype(np.float32))  # [D, B]
    w0 = np.asarray(route_weights).reshape(R, O, D, C)
    in_maps = []
    for i in range(NCORES):
        ws = w0[i * RLOC : (i + 1) * RLOC]  # (RLOC, O, D, C); r = 4*rq + g
        ws = ws.reshape(RG, NG, O, D, C).transpose(1, 3, 2, 4, 0)  # (g, d, o, c, rq)
        wprep = np.ascontiguousarray(ws.reshape(NG * D, O, C, RG).astype(np.float32))
        in_maps.append({"xt": xt, "w": wprep})
    return in_maps


def kernel(x, route_weights, trace=False):
    global LAST_EXEC_NS
    x = np.asarray(x, dtype=np.float32)
    route_weights = np.asarray(route_weights, dtype=np.float32)

    if "nc" not in _NC_CACHE:
        _NC_CACHE["nc"] = _build_nc()
    nc = _NC_CACHE["nc"]

    in_maps = _prep_inputs(x, route_weights)
    res = bass_utils.run_bass_kernel_spmd(
        nc, in_maps, core_ids=list(range(NCORES)), trace=trace
    )
    LAST_EXEC_NS = res.exec_time_ns

    shards = []
    for i in range(NCORES):
        o = res.results[i]["out"]  # [B, C, RLOC]
        shards.append(np.transpose(o, (0, 2, 1)))  # [B, RLOC, C]
    return np.concatenate(shards, axis=1).astype(np.float32)  # (B, R, C)
